# revision 1
# baseline (speedup 1.0000x reference)
"""Trainium2 Bass kernel for a dense recurrent scan (nn_CXBPU_55611236549128).

Math (per timestep t, K=4 microsteps):
    inj  = x_t @ W_in.T + b_in                  scattered into sensory_indices
    h    = relu(h @ W_rec.T + scatter(inj))     microstep 0
    h    = relu(h @ W_rec.T)                    microsteps 1..K-1
    out_t = h[:, output_indices] @ W_out.T + b_out

Sharding: data-parallel over batch, 8 rows per core, W_rec replicated.

Per-core design (feature-major "hT" layout [128 partitions, 16 chunks x 8 batch]):
  - W_rec.T resident in SBUF, streamed as the *moving* matmul operand every
    microstep (h-stationary keeps the weight transit on the fast streaming
    port instead of the 1.2 GHz LDWEIGHTS port).
  - Precision: W = W1 + W2 with both halves fp16 (exact 22-bit split; fp16
    subnormals are exact on the PE), h quantized to fp16 once per microstep
    by the relu write. Two fp16 passes accumulate in fp32 PSUM. End-to-end
    error vs fp32 reference ~4e-4 scale-relative absmax (the recurrence is
    contractive, spectral radius 0.9, so per-step quantization damps).
  - 4 k-tiles run concurrently in 4 PE column groups (tile_position=(0,32j)),
    issued column-group-innermost so the streams overlap.
  - A "transpose-sum" matmul against a 0/1 selector (i128) folds the 4
    partition groups back into feature-major hT for the next microstep
    (exact: fp16 values pass through fp32 PSUM untouched).
  - Injection is added as a host-precomputed dense tile already in hT layout.
  - Readout: 16 tiny matmuls vs scatter-expanded W_out (wsel).
"""

import os
from contextlib import ExitStack

import numpy as np

N = 2048
B = 64
T = 128
NCORES = 8
BPC = B // NCORES  # 8 batch rows per core
NCHUNK = N // 128  # 16

_CACHE = {}

# 'fp16x2' = two-pass fp16 split (fast), 'fp32' = exact fp32 (4-pass, slow)
MODE = os.environ.get("KERNEL_MM_MODE", "fp16x2")


def _build_nc(n_steps, mode=MODE):
    import concourse.bass as bass
    import concourse.mybir as mybir
    import concourse.tile as tile
    from concourse import bacc

    f32 = mybir.dt.float32
    f16 = mybir.dt.float16
    fmm = f16 if mode.startswith("fp16") else f32
    npass = 2 if mode == "fp16x2" else 1
    nc = bacc.Bacc(trn_type="TRN2")

    wt_d = nc.dram_tensor("wt", [npass * N, N], fmm, kind="ExternalInput")
    injd_d = nc.dram_tensor("injd", [n_steps, 128, 128], f32, kind="ExternalInput")
    wsel_d = nc.dram_tensor("wsel", [128, 2 * NCHUNK], fmm, kind="ExternalInput")
    i128_d = nc.dram_tensor("i128", [128, BPC], fmm, kind="ExternalInput")
    out_d = nc.dram_tensor("out", [2, n_steps * BPC], f32, kind="ExternalOutput")

    NSLAB = npass * NCHUNK

    with tile.TileContext(nc) as tc, ExitStack() as ctx:
        const = ctx.enter_context(tc.tile_pool(name="const", bufs=1))
        hpool = ctx.enter_context(tc.tile_pool(name="h", bufs=2))
        epool = ctx.enter_context(tc.tile_pool(name="evac", bufs=2))
        ipool = ctx.enter_context(tc.tile_pool(name="injd", bufs=2))
        ppool = ctx.enter_context(tc.tile_pool(name="psum", bufs=1, space="PSUM"))
        tpool = ctx.enter_context(tc.tile_pool(name="psumT", bufs=2, space="PSUM"))
        rpool = ctx.enter_context(tc.tile_pool(name="psumR", bufs=2, space="PSUM"))

        # resident W^T slabs: slab u = pass*16 + k-tile at cols [u*2048, ...).
        # Spread the 16 MB load across both HWDGE families + SWDGE.
        wt = const.tile([128, NSLAB * N], fmm)
        for u in range(NSLAB):
            eng = (nc.sync, nc.scalar, nc.gpsimd)[u % 3]
            eng.dma_start(wt[:, u * N : (u + 1) * N], wt_d[u * 128 : (u + 1) * 128, :])
        i128 = const.tile([128, BPC], fmm)
        nc.sync.dma_start(i128[:], i128_d[:])
        wsel = const.tile([128, 2 * NCHUNK], fmm)
        nc.sync.dma_start(wsel[:], wsel_d[:])
        outst = const.tile([2, n_steps * BPC], f32)

        psum = ppool.tile([128, N], f32)
        nc.vector.memset(psum[:], 0.0)

        hT = hpool.tile([128, NCHUNK * BPC], fmm)
        nc.vector.memset(hT[:], 0.0)

        tc.strict_bb_all_engine_barrier()

        # Tail work (transpose-sum + relu of banks 2-3, and the per-timestep
        # readout) is deferred into the NEXT microstep's instruction stream so
        # every PE wait lands >=2 banks after its producer. Legal because the
        # next microstep's round r only reads relu-bank r.
        pending = []

        for t in range(n_steps):
            injd = ipool.tile([128, 128], f32)
            nc.sync.dma_start(injd[:], injd_d[t])
            for s in range(4):
                # ---- main matmuls: psum[32j+b, n] += sum_k h[b,k] Wrec[n,k]
                # Bank-outer so bank n finishes early; its PSUM->SBUF evac and
                # transpose-sum matmuls then hide behind bank n+1's matmuls.
                # Within a bank, col-group j handles k-tiles {4r+j}; j
                # innermost so the 4 column-group streams overlap.
                evac = epool.tile([128, N], fmm)
                psumT = tpool.tile([128, NCHUNK * BPC], f32)

                def main_bank(n):
                    for r in range(4):
                        for p in range(npass):
                            for j in range(4):
                                kk = 4 * r + j
                                u = p * NCHUNK + kk
                                nc.tensor.matmul(
                                    psum[32 * j : 32 * j + BPC, 512 * n : 512 * (n + 1)],
                                    lhsT=hT[:, kk * BPC : (kk + 1) * BPC],
                                    rhs=wt[:, u * N + 512 * n : u * N + 512 * (n + 1)],
                                    start=(r == 0 and p == 0),
                                    stop=(r == 3 and p == npass - 1),
                                    tile_position=(0, 32 * j),
                                )

                def evac_bank(n):
                    # ACT copies cost ~2 us vs ~0.7 us on DVE; with 1-pass main
                    # matmuls the banks are too short to hide ACT, so keep all
                    # evacs on DVE there and alternate engines only for 2-pass.
                    if npass == 1 or n % 2 == 0:
                        nc.vector.tensor_copy(
                            evac[:, 512 * n : 512 * (n + 1)], psum[:, 512 * n : 512 * (n + 1)]
                        )
                    else:
                        nc.scalar.copy(
                            evac[:, 512 * n : 512 * (n + 1)], psum[:, 512 * n : 512 * (n + 1)]
                        )

                def tmm_bank(n, evac=evac, psumT=psumT):
                    # transpose-sum: psumT[m, c*8+b] = sum_j psum[32j+b, c*128+m]
                    for c in range(4 * n, 4 * n + 4):
                        nc.tensor.matmul(
                            psumT[:, c * BPC : (c + 1) * BPC],
                            lhsT=evac[:, c * 128 : (c + 1) * 128],
                            rhs=i128[:],
                            start=True,
                            stop=True,
                        )

                hT_new = hpool.tile([128, NCHUNK * BPC], fmm)

                def relu_bank(n, s=s, psumT=psumT, hT_new=hT_new, injd=injd):
                    # chunks 4n..4n+3 -> hT cols [32n, 32n+32); round r of the
                    # next microstep depends only on relu_bank(r).
                    cs = slice(32 * n, 32 * n + 32)
                    if s == 0:
                        nc.vector.tensor_add(hT_new[:, cs], psumT[:, cs], injd[:, cs])
                        nc.vector.tensor_relu(hT_new[:, cs], hT_new[:, cs])
                    else:
                        nc.vector.tensor_relu(hT_new[:, cs], psumT[:, cs])

                # flush deferred tail of the previous microstep first (its
                # relu banks 2-3 gate this microstep's rounds 2-3; its T-MMs
                # read an evac produced well before, so no PE wait).
                for fn in pending:
                    fn()
                pending = []

                main_bank(0)
                evac_bank(0)
                main_bank(1)
                evac_bank(1)
                tmm_bank(0)
                relu_bank(0)
                main_bank(2)
                evac_bank(2)
                tmm_bank(1)
                relu_bank(1)
                main_bank(3)
                evac_bank(3)
                pending = [
                    lambda n=2, f=tmm_bank: f(n),
                    lambda n=2, f=relu_bank: f(n),
                    lambda n=3, f=tmm_bank: f(n),
                    lambda n=3, f=relu_bank: f(n),
                ]
                hT = hT_new

            # ---- readout for timestep t from final hT (deferred behind the
            # pending relu banks 2-3 that complete that hT)
            def readout(t=t, hT=hT):
                pr = rpool.tile([2, BPC], f32)
                for c in range(NCHUNK):
                    nc.tensor.matmul(
                        pr[:],
                        lhsT=wsel[:, c * 2 : (c + 1) * 2],
                        rhs=hT[:, c * BPC : (c + 1) * BPC],
                        start=(c == 0),
                        stop=(c == NCHUNK - 1),
                    )
                nc.vector.tensor_copy(outst[:, t * BPC : (t + 1) * BPC], pr[:])

            pending.append(readout)

        for fn in pending:
            fn()
        nc.sync.dma_start(out_d[:], outst[:])
    nc.compile()
    return nc


def _prep_inputs(inputs, W_rec, W_in, b_in, W_out, sensory_indices, output_indices,
                 n_steps, mode=MODE):
    inputs = np.asarray(inputs, np.float32)
    W_rec = np.asarray(W_rec, np.float32)
    W_in = np.asarray(W_in, np.float32)
    b_in = np.asarray(b_in, np.float32)
    W_out = np.asarray(W_out, np.float32)
    sens = np.asarray(sensory_indices).astype(np.int64)
    oidx = np.asarray(output_indices).astype(np.int64)

    wtf = np.ascontiguousarray(W_rec.T)
    wsel_full = np.zeros((2, N), np.float32)
    np.add.at(wsel_full, (slice(None), oidx), W_out)
    wself = wsel_full.reshape(2, NCHUNK, 128).transpose(2, 1, 0).reshape(128, 2 * NCHUNK)

    if mode.startswith("fp16"):
        w1 = wtf.astype(np.float16)
        if mode == "fp16x2":
            w2 = (wtf - w1.astype(np.float32)).astype(np.float16)
            wt = np.ascontiguousarray(np.concatenate([w1, w2], axis=0))
        else:
            wt = np.ascontiguousarray(w1)
        wsel = np.ascontiguousarray(wself.astype(np.float16))
        i128 = (np.arange(128)[:, None] % 32 == np.arange(BPC)[None, :]).astype(np.float16)
    else:
        wt = wtf
        wsel = np.ascontiguousarray(wself)
        i128 = (np.arange(128)[:, None] % 32 == np.arange(BPC)[None, :]).astype(np.float32)

    # dense injection in hT layout, per core
    inj_all = inputs[:, :n_steps, :] @ W_in.T + b_in  # [B, T, 256]
    inj_dense = np.zeros((B, n_steps, N), np.float32)
    np.add.at(inj_dense, (slice(None), slice(None), sens), inj_all)
    injd_cores = []
    for g in range(NCORES):
        a = inj_dense[g * BPC : (g + 1) * BPC]  # [8, T, 2048]
        a = a.reshape(BPC, n_steps, NCHUNK, 128).transpose(1, 3, 2, 0)
        injd_cores.append(np.ascontiguousarray(a.reshape(n_steps, 128, NCHUNK * BPC)))

    return wt, injd_cores, wsel, i128


def _run(inputs, W_rec, W_in, b_in, W_out, b_out, sensory_indices, output_indices,
         K, n_steps=T, trace=False, mode=MODE):
    from concourse.bass_utils import run_bass_kernel_spmd

    assert int(K) == 4
    wt, injd_cores, wsel, i128 = _prep_inputs(
        inputs, W_rec, W_in, b_in, W_out, sensory_indices, output_indices,
        n_steps, mode)

    key = (n_steps, mode)
    if key not in _CACHE:
        _CACHE[key] = _build_nc(n_steps, mode)
    nc = _CACHE[key]

    in_maps = [
        {"wt": wt, "injd": injd_cores[g], "wsel": wsel, "i128": i128}
        for g in range(NCORES)
    ]
    res = run_bass_kernel_spmd(nc, in_maps, list(range(NCORES)), trace=trace)

    b_out = np.asarray(b_out, np.float32)
    outs = []
    for g in range(NCORES):
        r = np.asarray(res.results[g]["out"])  # [2, T*8]
        outs.append(r.reshape(2, n_steps, BPC).transpose(2, 1, 0))  # [8, T, 2]
    full = np.concatenate(outs, axis=0) + b_out  # [B, T, 2]
    return np.ascontiguousarray(full.astype(np.float32)), res


def kernel(**inputs):
    out, _ = _run(
        inputs["inputs"], inputs["W_rec"], inputs["W_in"], inputs["b_in"],
        inputs["W_out"], inputs["b_out"], inputs["sensory_indices"],
        inputs["output_indices"], inputs["K"],
    )
    return out



# revision 2
# speedup vs baseline: 1.0234x; 1.0234x over previous
"""Trainium2 Bass kernel for a dense recurrent scan (nn_CXBPU_55611236549128).

Math (per timestep t, K=4 microsteps):
    inj  = x_t @ W_in.T + b_in                  scattered into sensory_indices
    h    = relu(h @ W_rec.T + scatter(inj))     microstep 0
    h    = relu(h @ W_rec.T)                    microsteps 1..K-1
    out_t = h[:, output_indices] @ W_out.T + b_out

Sharding: data-parallel over batch, 8 rows per core, W_rec replicated.

Per-core design (v2 - "n-split" with DVE stream-transpose):
  - PE col group j owns output features [512j, 512j+512). Each group
    streams W^T for ALL 16 k-tiles and accumulates the complete h' for
    its quarter in one PSUM bank [32j+b, :512] -> no cross-group fold
    and no transpose-sum matmuls on the PE at all.
  - Precision: single-pass fp16 (W, h, x all fp16; fp32 PSUM accum).
    End-to-end norm-rel-err ~7e-4 (contractive recurrence damps the
    per-step h-quantization error).
  - h lives in a "scrambled hT" layout produced by ONE DVE
    StreamTranspose per phase: 32x32 blocks of the batch-major evac are
    transposed in place, so partition p of k-tile tt holds feature
    pi_tt(p) = 512*(p//32) + 32*tt + (p%32). The host pre-permutes the
    W^T slab rows (and wsel rows) with the same pi, making the scramble
    free.
  - Microstep columns are processed in 2 phases (320 + 192 cols) so the
    relu(ACT) + transpose(DVE) tail of each phase hides under the
    remaining matmul stream; next microstep's k-order (ascending) then
    never waits: phase-1 tiles are ready before its slot 0, phase-2
    tiles before slot 10.
  - Injection is a K=5 matmul (4 input dims + a constant-1 bias row)
    accumulated directly into PSUM: no per-timestep DMA.
  - Readout: 16 tiny N=2 matmuls reusing the main loop's stationaries,
    4 per col group into psum_r[32j+b, 0:2]; host folds the 4 group
    partials and adds b_out.
"""

from contextlib import ExitStack

import numpy as np

N = 2048
B = 64
T = 128
NCORES = 8
BPC = B // NCORES  # 8 batch rows per core
NT = 16  # k-tiles of 128 features
PHASES = (10, 6)  # k-tile split per phase; cols = 32*ntile (320 + 192)

_CACHE = {}


def _tile_perm():
    """perm[tt, p] = source feature held at partition p of k-tile tt."""
    p = np.arange(128)
    return (512 * (p[None, :] // 32) + 32 * np.arange(NT)[:, None] + (p[None, :] % 32))


def _build_nc(n_steps):
    import concourse.mybir as mybir
    import concourse.tile as tile
    from concourse import bacc

    f32 = mybir.dt.float32
    f16 = mybir.dt.float16
    relu = mybir.ActivationFunctionType.Relu
    nc = bacc.Bacc(trn_type="TRN2")

    wt_d = nc.dram_tensor("wt", [NT * 128, N], f16, kind="ExternalInput")
    xt_d = nc.dram_tensor("xt", [5, n_steps * BPC], f16, kind="ExternalInput")
    xw_d = nc.dram_tensor("xw", [5, N], f16, kind="ExternalInput")
    wsel_d = nc.dram_tensor("wsel", [128, 2 * NT], f16, kind="ExternalInput")
    out_d = nc.dram_tensor("out", [128, 2 * n_steps], f32, kind="ExternalOutput")

    with tile.TileContext(nc) as tc, ExitStack() as ctx:
        const = ctx.enter_context(tc.tile_pool(name="const", bufs=1))
        hpool = ctx.enter_context(tc.tile_pool(name="h", bufs=3))
        bmpool = ctx.enter_context(tc.tile_pool(name="bm", bufs=3))
        # one PSUM pool per phase: phase-2's start=True lands in a different
        # bank than the one phase-1's evac is still reading, so the scheduler
        # never serializes the in-order PE queue on the bank hazard
        ppoolA = ctx.enter_context(tc.tile_pool(name="psumA", bufs=2, space="PSUM"))
        ppoolB = ctx.enter_context(tc.tile_pool(name="psumB", bufs=2, space="PSUM"))
        rpool = ctx.enter_context(tc.tile_pool(name="psumR", bufs=2, space="PSUM"))

        # resident W^T slabs (row-permuted per k-tile): tile tt at cols
        # [tt*N, (tt+1)*N). Spread the 8 MB load across DMA families.
        wt = const.tile([128, NT * N], f16)
        for u in range(NT):
            eng = (nc.sync, nc.scalar, nc.gpsimd)[u % 3]
            eng.dma_start(wt[:, u * N : (u + 1) * N], wt_d[u * 128 : (u + 1) * 128, :])
        xt = const.tile([5, n_steps * BPC], f16)
        nc.sync.dma_start(xt[:], xt_d[:])
        xw = const.tile([5, N], f16)
        nc.sync.dma_start(xw[:], xw_d[:])
        wsel = const.tile([128, 2 * NT], f16)
        nc.sync.dma_start(wsel[:], wsel_d[:])
        outst = const.tile([128, 2 * n_steps], f32)

        hT = hpool.tile([128, 512], f16)  # scrambled hT; tile tt batch at cols 32tt..+8
        nc.vector.memset(hT[:], 0.0)

        tc.strict_bb_all_engine_barrier()

        n_micro = n_steps * 4
        PIECES = ((128, 192), (192,))  # tail col pieces per phase
        for step in range(n_micro):
            t, s = divmod(step, 4)
            psA = ppoolA.tile([128, 32 * PHASES[0]], f32)
            psB = ppoolB.tile([128, 32 * PHASES[1]], f32)
            psums = [psA, psB]
            h_bm = bmpool.tile([128, 512], f16)
            hT_new = hpool.tile([128, 512], f16)
            if s == 0 and t > 0:
                psum_r = rpool.tile([128, 2], f32)

            col0 = 0
            for phase, ntile in enumerate(PHASES):
                w = 32 * ntile
                psum = psums[phase]
                for tt in range(NT):
                    for j in range(4):
                        if s == 0 and tt == 0:
                            # injection: psum[b, n] += sum_i x[b,i]*Wsc[i,n] (+bias row)
                            nc.tensor.matmul(
                                psum[32 * j : 32 * j + BPC, 0:w],
                                lhsT=xt[:, t * BPC : (t + 1) * BPC],
                                rhs=xw[:, 512 * j + col0 : 512 * j + col0 + w],
                                start=True,
                                stop=False,
                                tile_position=(0, 32 * j),
                            )
                        nc.tensor.matmul(
                            psum[32 * j : 32 * j + BPC, 0:w],
                            lhsT=hT[:, 32 * tt : 32 * tt + BPC],
                            rhs=wt[:, tt * N + 512 * j + col0 : tt * N + 512 * j + col0 + w],
                            start=(tt == 0 and s != 0),
                            stop=(tt == NT - 1),
                            tile_position=(0, 32 * j),
                        )
                    if phase == 0 and s == 0 and t > 0:
                        # readout of timestep t-1 from its final h (= this
                        # microstep's stationaries); group tt%4.
                        jr = tt % 4
                        nc.tensor.matmul(
                            psum_r[32 * jr : 32 * jr + BPC, 0:2],
                            lhsT=hT[:, 32 * tt : 32 * tt + BPC],
                            rhs=wsel[:, 2 * tt : 2 * tt + 2],
                            start=(tt < 4),
                            stop=(tt >= NT - 4),
                            tile_position=(0, 32 * jr),
                        )
                # tail: relu+cast evac on ACT, 32x32 block transpose on DVE,
                # staged in column pieces so early k-tiles are ready sooner
                off = 0
                for pw in PIECES[phase]:
                    nc.scalar.activation(
                        h_bm[:, col0 + off : col0 + off + pw],
                        psum[:, off : off + pw], relu)
                    nc.vector.transpose(
                        hT_new[:, col0 + off : col0 + off + pw],
                        h_bm[:, col0 + off : col0 + off + pw])
                    off += pw
                col0 += w

            if s == 0 and t > 0:
                nc.vector.tensor_copy(outst[:, 2 * (t - 1) : 2 * t], psum_r[:])
            hT = hT_new

        # ghost readout for the last timestep
        psum_r = rpool.tile([128, 2], f32)
        for tt in range(NT):
            jr = tt % 4
            nc.tensor.matmul(
                psum_r[32 * jr : 32 * jr + BPC, 0:2],
                lhsT=hT[:, 32 * tt : 32 * tt + BPC],
                rhs=wsel[:, 2 * tt : 2 * tt + 2],
                start=(tt < 4),
                stop=(tt >= NT - 4),
                tile_position=(0, 32 * jr),
            )
        nc.vector.tensor_copy(outst[:, 2 * (n_steps - 1) : 2 * n_steps], psum_r[:])

        nc.sync.dma_start(out_d[:], outst[:])
    nc.compile()
    return nc


def _prep_inputs(inputs, W_rec, W_in, b_in, W_out, sensory_indices, output_indices,
                 n_steps):
    inputs = np.asarray(inputs, np.float32)
    W_rec = np.asarray(W_rec, np.float32)
    W_in = np.asarray(W_in, np.float32)
    b_in = np.asarray(b_in, np.float32)
    W_out = np.asarray(W_out, np.float32)
    sens = np.asarray(sensory_indices).astype(np.int64)
    oidx = np.asarray(output_indices).astype(np.int64)

    perm = _tile_perm()  # [NT, 128]
    wrec_t = np.ascontiguousarray(W_rec.T)  # [k, n]
    wt = np.ascontiguousarray(
        wrec_t[perm.reshape(-1), :].astype(np.float16))  # [NT*128, N]

    # injection weights: Wsc[i, n] = scatter of W_in.T; row 4 = scatter of b_in
    xw = np.zeros((5, N), np.float32)
    np.add.at(xw.T, (sens, slice(None)),
              np.concatenate([W_in, b_in[:, None]], axis=1).astype(np.float32))
    xw = np.ascontiguousarray(xw.astype(np.float16))

    # readout weights, row-permuted per k-tile: wsel[p, 2t+o] = Wsel[o, perm[t, p]]
    wsel_full = np.zeros((2, N), np.float32)
    np.add.at(wsel_full, (slice(None), oidx), W_out)
    wsel = np.ascontiguousarray(
        wsel_full[:, perm].transpose(2, 1, 0).reshape(128, NT * 2).astype(np.float16))

    # per-core transposed inputs with constant-1 bias row: xt[i, t*8+b]
    xt_cores = []
    for g in range(NCORES):
        xc = inputs[g * BPC : (g + 1) * BPC, :n_steps, :]  # [8, T, 4]
        xt = np.concatenate(
            [xc.transpose(2, 1, 0).reshape(4, n_steps * BPC),
             np.ones((1, n_steps * BPC), np.float32)], axis=0)
        xt_cores.append(np.ascontiguousarray(xt.astype(np.float16)))

    return wt, xt_cores, xw, wsel


def _run(inputs, W_rec, W_in, b_in, W_out, b_out, sensory_indices, output_indices,
         K, n_steps=T, trace=False):
    from concourse.bass_utils import run_bass_kernel_spmd

    assert int(K) == 4
    wt, xt_cores, xw, wsel = _prep_inputs(
        inputs, W_rec, W_in, b_in, W_out, sensory_indices, output_indices, n_steps)

    if n_steps not in _CACHE:
        _CACHE[n_steps] = _build_nc(n_steps)
    nc = _CACHE[n_steps]

    in_maps = [
        {"wt": wt, "xt": xt_cores[g], "xw": xw, "wsel": wsel}
        for g in range(NCORES)
    ]
    res = run_bass_kernel_spmd(nc, in_maps, list(range(NCORES)), trace=trace)

    b_out = np.asarray(b_out, np.float32)
    outs = []
    for g in range(NCORES):
        r = np.asarray(res.results[g]["out"])  # [128, 2T]; fold 4 group partials
        r4 = r.reshape(4, 32, 2 * n_steps)[:, :BPC, :].sum(axis=0)  # [8, 2T]
        outs.append(r4.reshape(BPC, n_steps, 2))
    full = np.concatenate(outs, axis=0) + b_out  # [B, T, 2]
    return np.ascontiguousarray(full.astype(np.float32)), res


def kernel(**inputs):
    out, _ = _run(
        inputs["inputs"], inputs["W_rec"], inputs["W_in"], inputs["b_in"],
        inputs["W_out"], inputs["b_out"], inputs["sensory_indices"],
        inputs["output_indices"], inputs["K"],
    )
    return out


# revision 4
# speedup vs baseline: 1.0274x; 1.0039x over previous
"""Trainium2 Bass kernel for a dense recurrent scan (nn_CXBPU_55611236549128).

Math (per timestep t, K=4 microsteps):
    inj  = x_t @ W_in.T + b_in                  scattered into sensory_indices
    h    = relu(h @ W_rec.T + scatter(inj))     microstep 0
    h    = relu(h @ W_rec.T)                    microsteps 1..K-1
    out_t = h[:, output_indices] @ W_out.T + b_out

Sharding: data-parallel over batch, 8 rows per core, W_rec replicated.

Per-core design (v2 - "n-split" with DVE stream-transpose):
  - PE col group j owns output features [512j, 512j+512). Each group
    streams W^T for ALL 16 k-tiles and accumulates the complete h' for
    its quarter in one PSUM bank [32j+b, :512] -> no cross-group fold
    and no transpose-sum matmuls on the PE at all.
  - Precision: single-pass fp16 (W, h, x all fp16; fp32 PSUM accum).
    End-to-end norm-rel-err ~7e-4 (contractive recurrence damps the
    per-step h-quantization error).
  - h lives in a "scrambled hT" layout produced by ONE DVE
    StreamTranspose per phase: 32x32 blocks of the batch-major evac are
    transposed in place, so partition p of k-tile tt holds feature
    pi_tt(p) = 512*(p//32) + 32*tt + (p%32). The host pre-permutes the
    W^T slab rows (and wsel rows) with the same pi, making the scramble
    free.
  - Microstep columns are processed in 2 phases (320 + 192 cols) so the
    relu + transpose tail of each phase hides under the remaining
    matmul stream; next microstep's k-order (ascending) then never
    waits: phase-1 tiles are ready before its slot 0, phase-2 tiles
    before slot 10. The tail runs entirely on the Vector engine
    (tensor_relu from PSUM, then StreamTranspose) - same-engine FIFO
    chaining avoids a cross-engine semaphore hop per piece.
  - Steady-state is LDWEIGHTS-issue-bound: 128 weight loads/microstep
    (16 tiles x 2 phases x 4 groups) x ~34ns ~= 4.37us/microstep, just
    above the 3.41us matmul-streaming floor.
  - Injection is a K=5 matmul (4 input dims + a constant-1 bias row)
    accumulated directly into PSUM: no per-timestep DMA.
  - Readout: 16 tiny N=2 matmuls reusing the main loop's stationaries,
    4 per col group into psum_r[32j+b, 0:2]; host folds the 4 group
    partials and adds b_out.
"""

from contextlib import ExitStack

import numpy as np

N = 2048
B = 64
T = 128
NCORES = 8
BPC = B // NCORES  # 8 batch rows per core
NT = 16  # k-tiles of 128 features
PHASES = (10, 6)  # k-tile split per phase; cols = 32*ntile (320 + 192)

_CACHE = {}


def _tile_perm():
    """perm[tt, p] = source feature held at partition p of k-tile tt."""
    p = np.arange(128)
    return (512 * (p[None, :] // 32) + 32 * np.arange(NT)[:, None] + (p[None, :] % 32))


def _build_nc(n_steps):
    import concourse.mybir as mybir
    import concourse.tile as tile
    from concourse import bacc

    f32 = mybir.dt.float32
    f16 = mybir.dt.float16
    relu = mybir.ActivationFunctionType.Relu
    nc = bacc.Bacc(trn_type="TRN2")

    wt_d = nc.dram_tensor("wt", [NT * 128, N], f16, kind="ExternalInput")
    xt_d = nc.dram_tensor("xt", [5, n_steps * BPC], f16, kind="ExternalInput")
    xw_d = nc.dram_tensor("xw", [5, N], f16, kind="ExternalInput")
    wsel_d = nc.dram_tensor("wsel", [128, 2 * NT], f16, kind="ExternalInput")
    out_d = nc.dram_tensor("out", [128, 2 * n_steps], f32, kind="ExternalOutput")

    with tile.TileContext(nc) as tc, ExitStack() as ctx:
        const = ctx.enter_context(tc.tile_pool(name="const", bufs=1))
        hpool = ctx.enter_context(tc.tile_pool(name="h", bufs=3))
        bmpool = ctx.enter_context(tc.tile_pool(name="bm", bufs=3))
        # one PSUM pool per phase: phase-2's start=True lands in a different
        # bank than the one phase-1's evac is still reading, so the scheduler
        # never serializes the in-order PE queue on the bank hazard
        ppoolA = ctx.enter_context(tc.tile_pool(name="psumA", bufs=2, space="PSUM"))
        ppoolB = ctx.enter_context(tc.tile_pool(name="psumB", bufs=2, space="PSUM"))
        rpool = ctx.enter_context(tc.tile_pool(name="psumR", bufs=2, space="PSUM"))

        # resident W^T slabs (row-permuted per k-tile): tile tt at cols
        # [tt*N, (tt+1)*N). Spread the 8 MB load across DMA families.
        wt = const.tile([128, NT * N], f16)
        for u in range(NT):
            eng = (nc.sync, nc.scalar, nc.gpsimd)[u % 3]
            eng.dma_start(wt[:, u * N : (u + 1) * N], wt_d[u * 128 : (u + 1) * 128, :])
        xt = const.tile([5, n_steps * BPC], f16)
        nc.sync.dma_start(xt[:], xt_d[:])
        xw = const.tile([5, N], f16)
        nc.sync.dma_start(xw[:], xw_d[:])
        wsel = const.tile([128, 2 * NT], f16)
        nc.sync.dma_start(wsel[:], wsel_d[:])
        outst = const.tile([128, 2 * n_steps], f32)

        hT = hpool.tile([128, 512], f16)  # scrambled hT; tile tt batch at cols 32tt..+8
        nc.vector.memset(hT[:], 0.0)

        tc.strict_bb_all_engine_barrier()

        n_micro = n_steps * 4
        PIECES = ((128, 192), (192,))  # tail col pieces per phase
        for step in range(n_micro):
            t, s = divmod(step, 4)
            psA = ppoolA.tile([128, 32 * PHASES[0]], f32)
            psB = ppoolB.tile([128, 32 * PHASES[1]], f32)
            psums = [psA, psB]
            h_bm = bmpool.tile([128, 512], f16)
            hT_new = hpool.tile([128, 512], f16)
            if s == 0 and t > 0:
                # full-bank tile so consecutive readout groups land in
                # different PSUM banks (a [128,2] tile would share one)
                psum_r = rpool.tile([128, 512], f32)

            col0 = 0
            for phase, ntile in enumerate(PHASES):
                w = 32 * ntile
                psum = psums[phase]
                for tt in range(NT):
                    for j in range(4):
                        if s == 0 and tt == 0:
                            # injection: psum[b, n] += sum_i x[b,i]*Wsc[i,n] (+bias row)
                            nc.tensor.matmul(
                                psum[32 * j : 32 * j + BPC, 0:w],
                                lhsT=xt[:, t * BPC : (t + 1) * BPC],
                                rhs=xw[:, 512 * j + col0 : 512 * j + col0 + w],
                                start=True,
                                stop=False,
                                tile_position=(0, 32 * j),
                            )
                        nc.tensor.matmul(
                            psum[32 * j : 32 * j + BPC, 0:w],
                            lhsT=hT[:, 32 * tt : 32 * tt + BPC],
                            rhs=wt[:, tt * N + 512 * j + col0 : tt * N + 512 * j + col0 + w],
                            start=(tt == 0 and s != 0),
                            stop=(tt == NT - 1),
                            tile_position=(0, 32 * j),
                        )
                # tail entirely on DVE (relu+cast straight from PSUM, then
                # 32x32 block transpose): same-engine FIFO chaining avoids a
                # cross-engine semaphore hop per piece (~270ns each here).
                # Staged in column pieces so early k-tiles are ready sooner.
                off = 0
                for pw in PIECES[phase]:
                    nc.vector.tensor_relu(
                        h_bm[:, col0 + off : col0 + off + pw],
                        psum[:, off : off + pw])
                    nc.vector.transpose(
                        hT_new[:, col0 + off : col0 + off + pw],
                        h_bm[:, col0 + off : col0 + off + pw])
                    off += pw
                if phase == 0 and s == 0 and t > 0:
                    # readout of timestep t-1 from its final h (= this
                    # microstep's stationaries), between the phases where the
                    # PE stream has maximal slack; group tt%4.
                    for tt in range(NT):
                        jr = tt % 4
                        nc.tensor.matmul(
                            psum_r[32 * jr : 32 * jr + BPC, 0:2],
                            lhsT=hT[:, 32 * tt : 32 * tt + BPC],
                            rhs=wsel[:, 2 * tt : 2 * tt + 2],
                            start=(tt < 4),
                            stop=(tt >= NT - 4),
                            tile_position=(0, 32 * jr),
                        )
                col0 += w

            if s == 0 and t > 0:
                nc.vector.tensor_copy(outst[:, 2 * (t - 1) : 2 * t], psum_r[:, 0:2])
            hT = hT_new

        # ghost readout for the last timestep
        psum_r = rpool.tile([128, 512], f32)
        for tt in range(NT):
            jr = tt % 4
            nc.tensor.matmul(
                psum_r[32 * jr : 32 * jr + BPC, 0:2],
                lhsT=hT[:, 32 * tt : 32 * tt + BPC],
                rhs=wsel[:, 2 * tt : 2 * tt + 2],
                start=(tt < 4),
                stop=(tt >= NT - 4),
                tile_position=(0, 32 * jr),
            )
        nc.vector.tensor_copy(outst[:, 2 * (n_steps - 1) : 2 * n_steps], psum_r[:, 0:2])

        nc.sync.dma_start(out_d[:], outst[:])
    nc.compile()
    return nc


def _prep_inputs(inputs, W_rec, W_in, b_in, W_out, sensory_indices, output_indices,
                 n_steps):
    inputs = np.asarray(inputs, np.float32)
    W_rec = np.asarray(W_rec, np.float32)
    W_in = np.asarray(W_in, np.float32)
    b_in = np.asarray(b_in, np.float32)
    W_out = np.asarray(W_out, np.float32)
    sens = np.asarray(sensory_indices).astype(np.int64)
    oidx = np.asarray(output_indices).astype(np.int64)

    perm = _tile_perm()  # [NT, 128]
    wrec_t = np.ascontiguousarray(W_rec.T)  # [k, n]
    wt = np.ascontiguousarray(
        wrec_t[perm.reshape(-1), :].astype(np.float16))  # [NT*128, N]

    # injection weights: Wsc[i, n] = scatter of W_in.T; row 4 = scatter of b_in
    xw = np.zeros((5, N), np.float32)
    np.add.at(xw.T, (sens, slice(None)),
              np.concatenate([W_in, b_in[:, None]], axis=1).astype(np.float32))
    xw = np.ascontiguousarray(xw.astype(np.float16))

    # readout weights, row-permuted per k-tile: wsel[p, 2t+o] = Wsel[o, perm[t, p]]
    wsel_full = np.zeros((2, N), np.float32)
    np.add.at(wsel_full, (slice(None), oidx), W_out)
    wsel = np.ascontiguousarray(
        wsel_full[:, perm].transpose(2, 1, 0).reshape(128, NT * 2).astype(np.float16))

    # per-core transposed inputs with constant-1 bias row: xt[i, t*8+b]
    xt_cores = []
    for g in range(NCORES):
        xc = inputs[g * BPC : (g + 1) * BPC, :n_steps, :]  # [8, T, 4]
        xt = np.concatenate(
            [xc.transpose(2, 1, 0).reshape(4, n_steps * BPC),
             np.ones((1, n_steps * BPC), np.float32)], axis=0)
        xt_cores.append(np.ascontiguousarray(xt.astype(np.float16)))

    return wt, xt_cores, xw, wsel


def _run(inputs, W_rec, W_in, b_in, W_out, b_out, sensory_indices, output_indices,
         K, n_steps=T, trace=False):
    from concourse.bass_utils import run_bass_kernel_spmd

    assert int(K) == 4
    wt, xt_cores, xw, wsel = _prep_inputs(
        inputs, W_rec, W_in, b_in, W_out, sensory_indices, output_indices, n_steps)

    if n_steps not in _CACHE:
        _CACHE[n_steps] = _build_nc(n_steps)
    nc = _CACHE[n_steps]

    in_maps = [
        {"wt": wt, "xt": xt_cores[g], "xw": xw, "wsel": wsel}
        for g in range(NCORES)
    ]
    res = run_bass_kernel_spmd(nc, in_maps, list(range(NCORES)), trace=trace)

    b_out = np.asarray(b_out, np.float32)
    outs = []
    for g in range(NCORES):
        r = np.asarray(res.results[g]["out"])  # [128, 2T]; fold 4 group partials
        r4 = r.reshape(4, 32, 2 * n_steps)[:, :BPC, :].sum(axis=0)  # [8, 2T]
        outs.append(r4.reshape(BPC, n_steps, 2))
    full = np.concatenate(outs, axis=0) + b_out  # [B, T, 2]
    return np.ascontiguousarray(full.astype(np.float32)), res


def kernel(**inputs):
    out, _ = _run(
        inputs["inputs"], inputs["W_rec"], inputs["W_in"], inputs["b_in"],
        inputs["W_out"], inputs["b_out"], inputs["sensory_indices"],
        inputs["output_indices"], inputs["K"],
    )
    return out


# revision 5
# speedup vs baseline: 1.1317x; 1.1015x over previous
"""Trainium2 Bass kernel for a dense recurrent scan (nn_CXBPU_55611236549128).

Math (per timestep t, K=4 microsteps):
    inj  = x_t @ W_in.T + b_in                  scattered into sensory_indices
    h    = relu(h @ W_rec.T + scatter(inj))     microstep 0
    h    = relu(h @ W_rec.T)                    microsteps 1..K-1
    out_t = h[:, output_indices] @ W_out.T + b_out

Sharding: data-parallel over batch, 8 rows per core, W_rec replicated.

Per-core design (v2 - "n-split" with DVE stream-transpose):
  - PE col group j owns output features [512j, 512j+512). Each group
    streams W^T for ALL 16 k-tiles and accumulates the complete h' for
    its quarter in one PSUM bank [32j+b, :512] -> no cross-group fold
    and no transpose-sum matmuls on the PE at all.
  - Precision: single-pass fp16 (W, h, x all fp16; fp32 PSUM accum).
    End-to-end norm-rel-err ~7e-4 (contractive recurrence damps the
    per-step h-quantization error).
  - h lives in a "scrambled hT" layout produced by ONE DVE
    StreamTranspose per phase: 32x32 blocks of the batch-major evac are
    transposed in place, so partition p of k-tile tt holds feature
    pi_tt(p) = 512*(p//32) + 32*tt + (p%32). The host pre-permutes the
    W^T slab rows (and wsel rows) with the same pi, making the scramble
    free.
  - Microstep columns are processed in 2 phases (320 + 192 cols) so the
    relu + transpose tail of each phase hides under the remaining
    matmul stream; next microstep's k-order (ascending) then never
    waits: phase-1 tiles are ready before its slot 0, phase-2 tiles
    before slot 10. The tail runs entirely on the Vector engine
    (tensor_relu from PSUM, then StreamTranspose) - same-engine FIFO
    chaining avoids a cross-engine semaphore hop per piece.
  - Steady-state is LDWEIGHTS-issue-bound: 128 weight loads/microstep
    (16 tiles x 2 phases x 4 groups) x ~34ns ~= 4.37us/microstep, just
    above the 3.41us matmul-streaming floor.
  - Injection is a K=5 matmul (4 input dims + a constant-1 bias row)
    accumulated directly into PSUM: no per-timestep DMA.
  - Readout: zero PE instructions - two DVE multiply+reduce pairs per
    timestep (per-partition dot of batch-major h against broadcast
    readout weights) run in the Vector engine's idle windows; host
    folds the 4 group partials and adds b_out.
"""

from contextlib import ExitStack

import numpy as np

N = 2048
B = 64
T = 128
NCORES = 8
BPC = B // NCORES  # 8 batch rows per core
NT = 16  # k-tiles of 128 features
PHASES = (10, 6)  # k-tile split per phase; cols = 32*ntile (320 + 192)

_CACHE = {}


def _tile_perm():
    """perm[tt, p] = source feature held at partition p of k-tile tt."""
    p = np.arange(128)
    return (512 * (p[None, :] // 32) + 32 * np.arange(NT)[:, None] + (p[None, :] % 32))


def _build_nc(n_steps):
    import concourse.mybir as mybir
    import concourse.tile as tile
    from concourse import bacc

    f32 = mybir.dt.float32
    f16 = mybir.dt.float16
    relu = mybir.ActivationFunctionType.Relu
    nc = bacc.Bacc(trn_type="TRN2")

    wt_d = nc.dram_tensor("wt", [NT * 128, N], f16, kind="ExternalInput")
    xt_d = nc.dram_tensor("xt", [5, n_steps * BPC], f16, kind="ExternalInput")
    xw_d = nc.dram_tensor("xw", [5, N], f16, kind="ExternalInput")
    wsel_d = nc.dram_tensor("wsel", [2 * 128, 512], f16, kind="ExternalInput")
    out_d = nc.dram_tensor("out", [128, 2 * n_steps], f32, kind="ExternalOutput")

    with tile.TileContext(nc) as tc, ExitStack() as ctx:
        const = ctx.enter_context(tc.tile_pool(name="const", bufs=1))
        hpool = ctx.enter_context(tc.tile_pool(name="h", bufs=3))
        bmpool = ctx.enter_context(tc.tile_pool(name="bm", bufs=3))
        # one PSUM pool per phase: phase-2's start=True lands in a different
        # bank than the one phase-1's evac is still reading, so the scheduler
        # never serializes the in-order PE queue on the bank hazard
        ppoolA = ctx.enter_context(tc.tile_pool(name="psumA", bufs=2, space="PSUM"))
        ppoolB = ctx.enter_context(tc.tile_pool(name="psumB", bufs=2, space="PSUM"))
        spool = ctx.enter_context(tc.tile_pool(name="rscr", bufs=2))

        # resident W^T slabs (row-permuted per k-tile): tile tt at cols
        # [tt*N, (tt+1)*N). Spread the 8 MB load across DMA families.
        wt = const.tile([128, NT * N], f16)
        for u in range(NT):
            eng = (nc.sync, nc.scalar, nc.gpsimd)[u % 3]
            eng.dma_start(wt[:, u * N : (u + 1) * N], wt_d[u * 128 : (u + 1) * 128, :])
        xt = const.tile([5, n_steps * BPC], f16)
        nc.sync.dma_start(xt[:], xt_d[:])
        xw = const.tile([5, N], f16)
        nc.sync.dma_start(xw[:], xw_d[:])
        wsel = const.tile([128, 2 * 512], f16)
        nc.sync.dma_start(wsel[:, 0:512], wsel_d[0:128, :])
        nc.sync.dma_start(wsel[:, 512:1024], wsel_d[128:256, :])
        outst = const.tile([128, 2 * n_steps], f32)

        hT = hpool.tile([128, 512], f16)  # scrambled hT; tile tt batch at cols 32tt..+8
        nc.vector.memset(hT[:], 0.0)

        tc.strict_bb_all_engine_barrier()

        n_micro = n_steps * 4
        PIECES = ((128, 192), (192,))  # tail col pieces per phase
        for step in range(n_micro):
            t, s = divmod(step, 4)
            if s == 0 and t > 0:
                h_bm_prev = h_bm  # batch-major final h of timestep t-1
            psA = ppoolA.tile([128, 32 * PHASES[0]], f32)
            psB = ppoolB.tile([128, 32 * PHASES[1]], f32)
            psums = [psA, psB]
            h_bm = bmpool.tile([128, 512], f16)
            hT_new = hpool.tile([128, 512], f16)

            col0 = 0
            for phase, ntile in enumerate(PHASES):
                w = 32 * ntile
                psum = psums[phase]
                for tt in range(NT):
                    for j in range(4):
                        if s == 0 and tt == 0:
                            # injection: psum[b, n] += sum_i x[b,i]*Wsc[i,n] (+bias row)
                            nc.tensor.matmul(
                                psum[32 * j : 32 * j + BPC, 0:w],
                                lhsT=xt[:, t * BPC : (t + 1) * BPC],
                                rhs=xw[:, 512 * j + col0 : 512 * j + col0 + w],
                                start=True,
                                stop=False,
                                tile_position=(0, 32 * j),
                            )
                        nc.tensor.matmul(
                            psum[32 * j : 32 * j + BPC, 0:w],
                            lhsT=hT[:, 32 * tt : 32 * tt + BPC],
                            rhs=wt[:, tt * N + 512 * j + col0 : tt * N + 512 * j + col0 + w],
                            start=(tt == 0 and s != 0),
                            stop=(tt == NT - 1),
                            tile_position=(0, 32 * j),
                        )
                # tail entirely on DVE (relu+cast straight from PSUM, then
                # 32x32 block transpose): same-engine FIFO chaining avoids a
                # cross-engine semaphore hop per piece (~270ns each here).
                # Staged in column pieces so early k-tiles are ready sooner.
                off = 0
                for pw in PIECES[phase]:
                    nc.vector.tensor_relu(
                        h_bm[:, col0 + off : col0 + off + pw],
                        psum[:, off : off + pw])
                    nc.vector.transpose(
                        hT_new[:, col0 + off : col0 + off + pw],
                        h_bm[:, col0 + off : col0 + off + pw])
                    off += pw
                col0 += w

            if s < 2 and t > 0:
                # readout of t-1 on DVE: outst[p, 2(t-1)+s] =
                #   sum_c h_bm_prev[p, c] * wsel[p, s*512+c]
                # (per-partition dot over the group's 512 cols; host folds
                # the 4 group partials). Emitted after the tails so the DVE
                # FIFO runs it in the idle window.
                rscr = spool.tile([128, 512], f16)
                nc.vector.tensor_mul(rscr[:], h_bm_prev[:], wsel[:, 512 * s : 512 * (s + 1)])
                nc.vector.tensor_reduce(
                    outst[:, 2 * (t - 1) + s : 2 * (t - 1) + s + 1], rscr[:],
                    mybir.AxisListType.X, mybir.AluOpType.add)
            hT = hT_new

        # ghost readout for the last timestep
        for o in range(2):
            rscr = spool.tile([128, 512], f16)
            nc.vector.tensor_mul(rscr[:], h_bm[:], wsel[:, 512 * o : 512 * (o + 1)])
            nc.vector.tensor_reduce(
                outst[:, 2 * (n_steps - 1) + o : 2 * (n_steps - 1) + o + 1], rscr[:],
                mybir.AxisListType.X, mybir.AluOpType.add)

        nc.sync.dma_start(out_d[:], outst[:])
    nc.compile()
    return nc


def _prep_inputs(inputs, W_rec, W_in, b_in, W_out, sensory_indices, output_indices,
                 n_steps):
    inputs = np.asarray(inputs, np.float32)
    W_rec = np.asarray(W_rec, np.float32)
    W_in = np.asarray(W_in, np.float32)
    b_in = np.asarray(b_in, np.float32)
    W_out = np.asarray(W_out, np.float32)
    sens = np.asarray(sensory_indices).astype(np.int64)
    oidx = np.asarray(output_indices).astype(np.int64)

    perm = _tile_perm()  # [NT, 128]
    wrec_t = np.ascontiguousarray(W_rec.T)  # [k, n]
    wt = np.ascontiguousarray(
        wrec_t[perm.reshape(-1), :].astype(np.float16))  # [NT*128, N]

    # injection weights: Wsc[i, n] = scatter of W_in.T; row 4 = scatter of b_in
    xw = np.zeros((5, N), np.float32)
    np.add.at(xw.T, (sens, slice(None)),
              np.concatenate([W_in, b_in[:, None]], axis=1).astype(np.float32))
    xw = np.ascontiguousarray(xw.astype(np.float16))

    # readout weights, batch-major: wsel[o*128+p, c] = Wsel[o, 512*(p//32)+c]
    wsel_full = np.zeros((2, N), np.float32)
    np.add.at(wsel_full, (slice(None), oidx), W_out)
    p = np.arange(128)
    wsel = np.ascontiguousarray(
        wsel_full[:, (512 * (p[:, None] // 32) + np.arange(512)[None, :])]
        .reshape(256, 512).astype(np.float16))

    # per-core transposed inputs with constant-1 bias row: xt[i, t*8+b]
    xt_cores = []
    for g in range(NCORES):
        xc = inputs[g * BPC : (g + 1) * BPC, :n_steps, :]  # [8, T, 4]
        xt = np.concatenate(
            [xc.transpose(2, 1, 0).reshape(4, n_steps * BPC),
             np.ones((1, n_steps * BPC), np.float32)], axis=0)
        xt_cores.append(np.ascontiguousarray(xt.astype(np.float16)))

    return wt, xt_cores, xw, wsel


def _run(inputs, W_rec, W_in, b_in, W_out, b_out, sensory_indices, output_indices,
         K, n_steps=T, trace=False):
    from concourse.bass_utils import run_bass_kernel_spmd

    assert int(K) == 4
    wt, xt_cores, xw, wsel = _prep_inputs(
        inputs, W_rec, W_in, b_in, W_out, sensory_indices, output_indices, n_steps)

    if n_steps not in _CACHE:
        _CACHE[n_steps] = _build_nc(n_steps)
    nc = _CACHE[n_steps]

    in_maps = [
        {"wt": wt, "xt": xt_cores[g], "xw": xw, "wsel": wsel}
        for g in range(NCORES)
    ]
    res = run_bass_kernel_spmd(nc, in_maps, list(range(NCORES)), trace=trace)

    b_out = np.asarray(b_out, np.float32)
    outs = []
    for g in range(NCORES):
        r = np.asarray(res.results[g]["out"])  # [128, 2T]; fold 4 group partials
        r4 = r.reshape(4, 32, 2 * n_steps)[:, :BPC, :].sum(axis=0)  # [8, 2T]
        outs.append(r4.reshape(BPC, n_steps, 2))
    full = np.concatenate(outs, axis=0) + b_out  # [B, T, 2]
    return np.ascontiguousarray(full.astype(np.float32)), res


def kernel(**inputs):
    out, _ = _run(
        inputs["inputs"], inputs["W_rec"], inputs["W_in"], inputs["b_in"],
        inputs["W_out"], inputs["b_out"], inputs["sensory_indices"],
        inputs["output_indices"], inputs["K"],
    )
    return out


# revision 6
# speedup vs baseline: 1.1323x; 1.0006x over previous
"""Trainium2 Bass kernel for a dense recurrent scan (nn_CXBPU_55611236549128).

Math (per timestep t, K=4 microsteps):
    inj  = x_t @ W_in.T + b_in                  scattered into sensory_indices
    h    = relu(h @ W_rec.T + scatter(inj))     microstep 0
    h    = relu(h @ W_rec.T)                    microsteps 1..K-1
    out_t = h[:, output_indices] @ W_out.T + b_out

Sharding: data-parallel over batch, 8 rows per core, W_rec replicated.

Per-core design (v2 - "n-split" with DVE stream-transpose):
  - PE col group j owns output features [512j, 512j+512). Each group
    streams W^T for ALL 16 k-tiles and accumulates the complete h' for
    its quarter in one PSUM bank [32j+b, :512] -> no cross-group fold
    and no transpose-sum matmuls on the PE at all.
  - Precision: single-pass fp16 (W, h, x all fp16; fp32 PSUM accum).
    End-to-end norm-rel-err ~7e-4 (contractive recurrence damps the
    per-step h-quantization error).
  - h lives in a "scrambled hT" layout produced by ONE DVE
    StreamTranspose per phase: 32x32 blocks of the batch-major evac are
    transposed in place, so partition p of k-tile tt holds feature
    pi_tt(p) = 512*(p//32) + 32*tt + (p%32). The host pre-permutes the
    W^T slab rows (and wsel rows) with the same pi, making the scramble
    free.
  - Microstep columns are processed in 2 phases (320 + 192 cols) so the
    relu + transpose tail of each phase hides under the remaining
    matmul stream; next microstep's k-order (ascending) then never
    waits: phase-1 tiles are ready before its slot 0, phase-2 tiles
    before slot 10. The tail runs entirely on the Vector engine
    (tensor_relu from PSUM, then StreamTranspose) - same-engine FIFO
    chaining avoids a cross-engine semaphore hop per piece.
  - Steady-state is LDWEIGHTS-issue-bound: 128 weight loads/microstep
    (16 tiles x 2 phases x 4 groups) x ~34ns ~= 4.37us/microstep, just
    above the 3.41us matmul-streaming floor.
  - Injection is a matmul (4 input dims + a constant-1 bias row,
    zero-padded to K=128 so it shares the mains' (128,32) PE tile
    config - mixed configs cost an array drain per switch) accumulated
    directly into PSUM: no per-timestep DMA.
  - Readout: 16 tiny N=2 matmuls reusing the main loop's stationaries,
    4 per col group into psum_r[32j+b, 0:2]; host folds the 4 group
    partials and adds b_out.
"""

from contextlib import ExitStack

import numpy as np

N = 2048
B = 64
T = 128
NCORES = 8
BPC = B // NCORES  # 8 batch rows per core
NT = 16  # k-tiles of 128 features
PHASES = (10, 6)  # k-tile split per phase; cols = 32*ntile (320 + 192)

_CACHE = {}


def _tile_perm():
    """perm[tt, p] = source feature held at partition p of k-tile tt."""
    p = np.arange(128)
    return (512 * (p[None, :] // 32) + 32 * np.arange(NT)[:, None] + (p[None, :] % 32))


def _build_nc(n_steps):
    import concourse.mybir as mybir
    import concourse.tile as tile
    from concourse import bacc

    f32 = mybir.dt.float32
    f16 = mybir.dt.float16
    relu = mybir.ActivationFunctionType.Relu
    nc = bacc.Bacc(trn_type="TRN2")

    wt_d = nc.dram_tensor("wt", [NT * 128, N], f16, kind="ExternalInput")
    xt_d = nc.dram_tensor("xt", [128, n_steps * BPC], f16, kind="ExternalInput")
    xw_d = nc.dram_tensor("xw", [128, N], f16, kind="ExternalInput")
    wsel_d = nc.dram_tensor("wsel", [2 * 128, 512], f16, kind="ExternalInput")
    out_d = nc.dram_tensor("out", [128, 2 * n_steps], f32, kind="ExternalOutput")

    with tile.TileContext(nc) as tc, ExitStack() as ctx:
        const = ctx.enter_context(tc.tile_pool(name="const", bufs=1))
        hpool = ctx.enter_context(tc.tile_pool(name="h", bufs=3))
        bmpool = ctx.enter_context(tc.tile_pool(name="bm", bufs=3))
        # one PSUM pool per phase: phase-2's start=True lands in a different
        # bank than the one phase-1's evac is still reading, so the scheduler
        # never serializes the in-order PE queue on the bank hazard
        ppoolA = ctx.enter_context(tc.tile_pool(name="psumA", bufs=2, space="PSUM"))
        ppoolB = ctx.enter_context(tc.tile_pool(name="psumB", bufs=2, space="PSUM"))
        spool = ctx.enter_context(tc.tile_pool(name="rscr", bufs=2))

        # resident W^T slabs (row-permuted per k-tile): tile tt at cols
        # [tt*N, (tt+1)*N). Spread the 8 MB load across DMA families.
        wt = const.tile([128, NT * N], f16)
        for u in range(NT):
            eng = (nc.sync, nc.scalar, nc.gpsimd)[u % 3]
            eng.dma_start(wt[:, u * N : (u + 1) * N], wt_d[u * 128 : (u + 1) * 128, :])
        xt = const.tile([128, n_steps * BPC], f16)
        nc.sync.dma_start(xt[:], xt_d[:])
        xw = const.tile([128, N], f16)
        nc.sync.dma_start(xw[:], xw_d[:])
        wsel = const.tile([128, 2 * 512], f16)
        nc.sync.dma_start(wsel[:, 0:512], wsel_d[0:128, :])
        nc.sync.dma_start(wsel[:, 512:1024], wsel_d[128:256, :])
        outst = const.tile([128, 2 * n_steps], f32)

        hT = hpool.tile([128, 512], f16)  # scrambled hT; tile tt batch at cols 32tt..+8
        nc.vector.memset(hT[:], 0.0)

        tc.strict_bb_all_engine_barrier()

        n_micro = n_steps * 4
        PIECES = ((128, 192), (192,))  # tail col pieces per phase
        for step in range(n_micro):
            t, s = divmod(step, 4)
            if s == 0 and t > 0:
                h_bm_prev = h_bm  # batch-major final h of timestep t-1
            psA = ppoolA.tile([128, 32 * PHASES[0]], f32)
            psB = ppoolB.tile([128, 32 * PHASES[1]], f32)
            psums = [psA, psB]
            h_bm = bmpool.tile([128, 512], f16)
            hT_new = hpool.tile([128, 512], f16)

            col0 = 0
            for phase, ntile in enumerate(PHASES):
                w = 32 * ntile
                psum = psums[phase]
                for tt in range(NT):
                    for j in range(4):
                        if s == 0 and tt == 0:
                            # injection: psum[b, n] += sum_i x[b,i]*Wsc[i,n] (+bias row)
                            nc.tensor.matmul(
                                psum[32 * j : 32 * j + BPC, 0:w],
                                lhsT=xt[:, t * BPC : (t + 1) * BPC],
                                rhs=xw[:, 512 * j + col0 : 512 * j + col0 + w],
                                start=True,
                                stop=False,
                                tile_position=(0, 32 * j),
                            )
                        nc.tensor.matmul(
                            psum[32 * j : 32 * j + BPC, 0:w],
                            lhsT=hT[:, 32 * tt : 32 * tt + BPC],
                            rhs=wt[:, tt * N + 512 * j + col0 : tt * N + 512 * j + col0 + w],
                            start=(tt == 0 and s != 0),
                            stop=(tt == NT - 1),
                            tile_position=(0, 32 * j),
                        )
                # tail entirely on DVE (relu+cast straight from PSUM, then
                # 32x32 block transpose): same-engine FIFO chaining avoids a
                # cross-engine semaphore hop per piece (~270ns each here).
                # Staged in column pieces so early k-tiles are ready sooner.
                off = 0
                for pw in PIECES[phase]:
                    nc.vector.tensor_relu(
                        h_bm[:, col0 + off : col0 + off + pw],
                        psum[:, off : off + pw])
                    nc.vector.transpose(
                        hT_new[:, col0 + off : col0 + off + pw],
                        h_bm[:, col0 + off : col0 + off + pw])
                    off += pw
                col0 += w

            if s < 2 and t > 0:
                # readout of t-1 on DVE: outst[p, 2(t-1)+s] =
                #   sum_c h_bm_prev[p, c] * wsel[p, s*512+c]
                # (per-partition dot over the group's 512 cols; host folds
                # the 4 group partials). Emitted after the tails so the DVE
                # FIFO runs it in the idle window.
                rscr = spool.tile([128, 512], f16)
                nc.vector.tensor_mul(rscr[:], h_bm_prev[:], wsel[:, 512 * s : 512 * (s + 1)])
                nc.vector.tensor_reduce(
                    outst[:, 2 * (t - 1) + s : 2 * (t - 1) + s + 1], rscr[:],
                    mybir.AxisListType.X, mybir.AluOpType.add)
            hT = hT_new

        # ghost readout for the last timestep
        for o in range(2):
            rscr = spool.tile([128, 512], f16)
            nc.vector.tensor_mul(rscr[:], h_bm[:], wsel[:, 512 * o : 512 * (o + 1)])
            nc.vector.tensor_reduce(
                outst[:, 2 * (n_steps - 1) + o : 2 * (n_steps - 1) + o + 1], rscr[:],
                mybir.AxisListType.X, mybir.AluOpType.add)

        nc.sync.dma_start(out_d[:], outst[:])
    nc.compile()
    return nc


def _prep_inputs(inputs, W_rec, W_in, b_in, W_out, sensory_indices, output_indices,
                 n_steps):
    inputs = np.asarray(inputs, np.float32)
    W_rec = np.asarray(W_rec, np.float32)
    W_in = np.asarray(W_in, np.float32)
    b_in = np.asarray(b_in, np.float32)
    W_out = np.asarray(W_out, np.float32)
    sens = np.asarray(sensory_indices).astype(np.int64)
    oidx = np.asarray(output_indices).astype(np.int64)

    perm = _tile_perm()  # [NT, 128]
    wrec_t = np.ascontiguousarray(W_rec.T)  # [k, n]
    wt = np.ascontiguousarray(
        wrec_t[perm.reshape(-1), :].astype(np.float16))  # [NT*128, N]

    # injection weights: Wsc[i, n] = scatter of W_in.T; row 4 = scatter of
    # b_in; zero-padded to K=128 so the injection matmul uses the same
    # (128,32) PE tile config as the mains (mixed configs force an array
    # drain per switch, ~225ns each, serializing the s=0 microstep)
    xw5 = np.zeros((5, N), np.float32)
    np.add.at(xw5.T, (sens, slice(None)),
              np.concatenate([W_in, b_in[:, None]], axis=1).astype(np.float32))
    xw = np.ascontiguousarray(
        np.vstack([xw5, np.zeros((123, N), np.float32)]).astype(np.float16))

    # readout weights, batch-major: wsel[o*128+p, c] = Wsel[o, 512*(p//32)+c]
    wsel_full = np.zeros((2, N), np.float32)
    np.add.at(wsel_full, (slice(None), oidx), W_out)
    p = np.arange(128)
    wsel = np.ascontiguousarray(
        wsel_full[:, (512 * (p[:, None] // 32) + np.arange(512)[None, :])]
        .reshape(256, 512).astype(np.float16))

    # per-core transposed inputs with constant-1 bias row: xt[i, t*8+b]
    xt_cores = []
    for g in range(NCORES):
        xc = inputs[g * BPC : (g + 1) * BPC, :n_steps, :]  # [8, T, 4]
        xt = np.concatenate(
            [xc.transpose(2, 1, 0).reshape(4, n_steps * BPC),
             np.ones((1, n_steps * BPC), np.float32),
             np.zeros((123, n_steps * BPC), np.float32)], axis=0)
        xt_cores.append(np.ascontiguousarray(xt.astype(np.float16)))

    return wt, xt_cores, xw, wsel


def _run(inputs, W_rec, W_in, b_in, W_out, b_out, sensory_indices, output_indices,
         K, n_steps=T, trace=False):
    from concourse.bass_utils import run_bass_kernel_spmd

    assert int(K) == 4
    wt, xt_cores, xw, wsel = _prep_inputs(
        inputs, W_rec, W_in, b_in, W_out, sensory_indices, output_indices, n_steps)

    if n_steps not in _CACHE:
        _CACHE[n_steps] = _build_nc(n_steps)
    nc = _CACHE[n_steps]

    in_maps = [
        {"wt": wt, "xt": xt_cores[g], "xw": xw, "wsel": wsel}
        for g in range(NCORES)
    ]
    res = run_bass_kernel_spmd(nc, in_maps, list(range(NCORES)), trace=trace)

    b_out = np.asarray(b_out, np.float32)
    outs = []
    for g in range(NCORES):
        r = np.asarray(res.results[g]["out"])  # [128, 2T]; fold 4 group partials
        r4 = r.reshape(4, 32, 2 * n_steps)[:, :BPC, :].sum(axis=0)  # [8, 2T]
        outs.append(r4.reshape(BPC, n_steps, 2))
    full = np.concatenate(outs, axis=0) + b_out  # [B, T, 2]
    return np.ascontiguousarray(full.astype(np.float32)), res


def kernel(**inputs):
    out, _ = _run(
        inputs["inputs"], inputs["W_rec"], inputs["W_in"], inputs["b_in"],
        inputs["W_out"], inputs["b_out"], inputs["sensory_indices"],
        inputs["output_indices"], inputs["K"],
    )
    return out
